# revision 1
# baseline (speedup 1.0000x reference)
"""Sparse attention (B=4,H=16,N=2048,D=64) on 8 trn2 NeuronCores.

Sharding: core c = bp*4 + hq handles batches [2bp, 2bp+1] x heads [4hq..4hq+3].
Per (b,h):  O^T = normalize( V~^T @ (P^T) ),  V~ = [V | 1]  (denominator for free)
  batch 0:  P^T = mask^T * exp(K Q^T/8 + bias^T)   (bias via identity-matmul in PSUM)
  batch 1:  P^T = (mask^T * exp(bias^T)) * exp(K Q^T/8)   (host-folded, streamed)
Adjacent key-tiles' S-matmuls run concurrently in PE row-groups 0-63/64-127
(q/k replicated in both partition halves; tile_position=(64*(kt%2), 0)).
Host does layout transforms and the final gather.
"""

import numpy as np
import ml_dtypes

import concourse.bass as bass
from concourse import bacc
import concourse.mybir as mybir
import concourse.tile as tile
from concourse.bass_utils import run_bass_kernel_spmd

dt = mybir.dt
AF = mybir.ActivationFunctionType

B, H, N, D = 4, 16, 2048, 64
NB = 2   # batches per core
NH = 4   # heads per core
P = 128
NKT = N // P          # 16 key tiles
QW = 512              # query tile width (one PSUM bank of fp32)
SUPW = 1024           # S supertile width (2 banks) -> fewer/larger ACT ops
NQP = N // SUPW       # 2 query supertiles
NQI = SUPW // QW      # 2 PSUM-bank columns per supertile
TRACE = False

_CACHE = {}


def build_bass():
    nc = bacc.Bacc()
    # q/k replicated in both partition halves: [NB, NH, 2D, N]
    qT = nc.declare_dram_parameter("qT", [NB, NH, 2 * D, N], dt.float16, isOutput=False)
    kT = nc.declare_dram_parameter("kT", [NB, NH, 2 * D, N], dt.float16, isOutput=False)
    vA = nc.declare_dram_parameter("vA", [NB, NH, N, D + 1], dt.float16, isOutput=False)
    maskT = nc.declare_dram_parameter("maskT", [1, N, N], dt.float16, isOutput=False)
    mex1 = nc.declare_dram_parameter("mex1", [NH, N, N], dt.float16, isOutput=False)
    biasT = nc.declare_dram_parameter("biasT", [NH, N, N], dt.float16, isOutput=False)
    ident = nc.declare_dram_parameter("ident", [P, P], dt.float16, isOutput=False)
    ones = nc.declare_dram_parameter("ones", [1, D], dt.float16, isOutput=False)
    outT = nc.declare_dram_parameter("outT", [NB, NH, D, N], dt.float32, isOutput=True)

    with tile.TileContext(nc) as tc:
        with (
            tc.tile_pool(name="const", bufs=1) as cpool,
            tc.tile_pool(name="mask", bufs=1) as mpool,
            tc.tile_pool(name="qk", bufs=2) as qkpool,
            tc.tile_pool(name="vp", bufs=2) as vpool,
            tc.tile_pool(name="bias", bufs=6) as bpool,
            tc.tile_pool(name="pt", bufs=4) as ppool,
            tc.tile_pool(name="norm", bufs=1) as rpool,
            tc.tile_pool(name="out", bufs=3) as opool_sb,
            tc.tile_pool(name="spsum", bufs=3, space="PSUM") as spool,
            tc.tile_pool(name="opsum", bufs=1, space="PSUM") as opool,
        ):
            ident_sb = cpool.tile([P, P], dt.float16, tag="ident")
            nc.gpsimd.dma_start(ident_sb, ident[:])
            ones_sb = cpool.tile([1, D], dt.float16, tag="ones")
            nc.gpsimd.dma_start(ones_sb, ones[:])

            # resident transposed mask for batch 0, chunks loaded at first use
            mask0 = mpool.tile([P, NKT, N], dt.float16, tag="mask0")

            for h in range(NH):
                qsb, ksb, vsb = [], [], []
                for b in range(NB):
                    qt_ = qkpool.tile([2 * D, N], dt.float16, tag=f"q{b}")
                    kt_ = qkpool.tile([2 * D, N], dt.float16, tag=f"k{b}")
                    if h == 0 and b == 0:
                        for j_ in range(0, NKT, 2):
                            nc.sync.dma_start(
                                kt_[:, j_ * P:(j_ + 2) * P],
                                kT[b, h, :, j_ * P:(j_ + 2) * P],
                            )
                        for qp_ in range(NQP):
                            nc.sync.dma_start(
                                qt_[:, qp_ * SUPW:(qp_ + 1) * SUPW],
                                qT[b, h, :, qp_ * SUPW:(qp_ + 1) * SUPW],
                            )
                    else:
                        nc.sync.dma_start(qt_, qT[b, h])
                        nc.sync.dma_start(kt_, kT[b, h])
                    vt_ = vpool.tile([P, NKT, D + 1], dt.float16, tag=f"v{b}")
                    nc.gpsimd.dma_start(vt_, vA[b, h].rearrange("(t p) c -> p t c", p=P))
                    qsb.append(qt_)
                    ksb.append(kt_)
                    vsb.append(vt_)

                for b in range(NB):
                    for qp in range(NQP):
                        q0 = qp * SUPW
                        opsum = []
                        for qi in range(NQI):
                            ot = opool.tile([D + 1, QW], dt.float32,
                                            tag=f"o{qi}", name=f"opsum{qi}")
                            opsum.append(ot)
                        for kt0 in range(0, NKT, 2):
                            ssups = []
                            pts = []
                            for j in range(2):
                                kt = kt0 + j
                                rg = (kt % 2) * D
                                src = biasT if b == 0 else mex1
                                bias_sb = bpool.tile([P, SUPW], dt.float16,
                                                     tag="bias", name=f"bias{j}")
                                nc.sync.dma_start(
                                    bias_sb,
                                    src[h, kt * P:(kt + 1) * P, q0:q0 + SUPW],
                                )
                                if h == 0 and b == 0 and qp == 0:
                                    nc.gpsimd.dma_start(
                                        mask0[:, kt],
                                        maskT[0, kt * P:(kt + 1) * P],
                                    )
                                ssup = spool.tile([P, SUPW], dt.float32,
                                                  tag="s", name=f"ssup{j}")
                                ssups.append((kt, rg, bias_sb, ssup))
                            # adjacent kt S-matmuls -> alternating row groups,
                            # issued pairwise for concurrency
                            for qi in range(NQI):
                                for kt, rg, _, ssup in ssups:
                                    nc.tensor.matmul(
                                        ssup[:, qi * QW:(qi + 1) * QW],
                                        ksb[b][rg:rg + D, kt * P:(kt + 1) * P],
                                        qsb[b][rg:rg + D,
                                               q0 + qi * QW:q0 + (qi + 1) * QW],
                                        start=True, stop=(b == 1),
                                        tile_position=(rg, 0),
                                    )
                            if b == 0:
                                for kt, rg, bias_sb, ssup in ssups:
                                    for qi in range(NQI):
                                        nc.tensor.matmul(
                                            ssup[:, qi * QW:(qi + 1) * QW],
                                            ident_sb,
                                            bias_sb[:, qi * QW:(qi + 1) * QW],
                                            start=False, stop=True,
                                        )
                            for kt, rg, bias_sb, ssup in ssups:
                                pt = ppool.tile([P, SUPW], dt.float16, tag="pt",
                                                name="pt")
                                nc.scalar.activation(pt, ssup, AF.Exp)
                                nc.vector.tensor_mul(
                                    pt, pt,
                                    mask0[:, kt, q0:q0 + SUPW] if b == 0
                                    else bias_sb,
                                )
                                pts.append((kt, pt))
                            for kt, pt in pts:
                                for qi in range(NQI):
                                    nc.tensor.matmul(
                                        opsum[qi],
                                        vsb[b][:, kt, :],
                                        pt[:, qi * QW:(qi + 1) * QW],
                                        start=(kt == 0), stop=(kt == NKT - 1),
                                    )
                        # drain O accumulators to SBUF fast (frees PSUM banks
                        # while the next tile's S-phase runs), normalize there
                        otmp = opool_sb.tile([D + 1, SUPW], dt.float32,
                                             tag="otmp", name="otmp")
                        for qi in range(NQI):
                            nc.scalar.copy(
                                otmp[:, qi * QW:(qi + 1) * QW], opsum[qi]
                            )
                        sums = rpool.tile([1, NQI * QW], dt.float32, tag="sums")
                        nc.vector.tensor_copy(sums, otmp[D:D + 1, :])
                        rec = rpool.tile([1, NQI * QW], dt.float32, tag="rec")
                        nc.vector.reciprocal_approx_fast(rec, sums)
                        rec16 = rpool.tile([1, NQI * QW], dt.float16, tag="rec16")
                        nc.vector.tensor_copy(rec16, rec)
                        rec64 = rpool.tile([D, NQI * QW], dt.float16, tag="rec64")
                        nc.gpsimd.partition_broadcast(rec64, rec16)
                        for qi in range(NQI):
                            osb = opool_sb.tile([D, QW], dt.float32, tag="osb")
                            nc.vector.tensor_mul(
                                osb, otmp[:D, qi * QW:(qi + 1) * QW],
                                rec64[:, qi * QW:(qi + 1) * QW],
                            )
                            nc.sync.dma_start(
                                outT[b, h, :, q0 + qi * QW:q0 + (qi + 1) * QW],
                                osb,
                            )
    nc.finalize()
    return nc


def make_in_maps(q, k, v, mask, attn_bias):
    scale = np.float32(D ** -0.5)
    qTf = (q.transpose(0, 1, 3, 2) * scale).astype(np.float16)   # [B,H,D,N]
    kTf = k.transpose(0, 1, 3, 2).astype(np.float16)
    vA = np.concatenate(
        [v, np.ones((B, H, N, 1), np.float32)], axis=-1
    ).astype(np.float16)                                         # [B,H,N,D+1]
    maskT = np.ascontiguousarray(
        mask[:, 0].transpose(0, 2, 1)
    ).astype(np.float16)                                         # [B,N,N] 0/1
    biasT32 = np.ascontiguousarray(
        attn_bias[0].transpose(0, 2, 1)
    )                                                            # [H,N,N] f32
    biasT = biasT32.astype(np.float16)
    expbT = np.exp(biasT32, dtype=np.float32).astype(np.float16)  # [H,N,N]
    ident = np.eye(P, dtype=np.float16)
    ones = np.ones((1, D), np.float16)

    in_maps = []
    for c in range(8):
        bp, hq = divmod(c, 4)
        bs, hs = 2 * bp, 4 * hq
        # replicate each batch's q/k into both partition halves: [NB,NH,2D,N]
        qrep = np.concatenate([qTf[bs:bs + NB, hs:hs + NH]] * 2, axis=2)
        krep = np.concatenate([kTf[bs:bs + NB, hs:hs + NH]] * 2, axis=2)
        in_maps.append({
            "qT": np.ascontiguousarray(qrep),
            "kT": np.ascontiguousarray(krep),
            "vA": np.ascontiguousarray(vA[bs:bs + NB, hs:hs + NH]),
            "maskT": np.ascontiguousarray(maskT[bs:bs + 1]),
            "mex1": np.ascontiguousarray(
                maskT[bs + 1][None] * expbT[hs:hs + NH]),
            "biasT": np.ascontiguousarray(biasT[hs:hs + NH]),
            "ident": ident,
            "ones": ones,
        })
    return in_maps


def kernel(q, k, v, mask, attn_bias):
    if "nc" not in _CACHE:
        _CACHE["nc"] = build_bass()
    nc = _CACHE["nc"]
    in_maps = make_in_maps(
        np.asarray(q, np.float32), np.asarray(k, np.float32),
        np.asarray(v, np.float32), np.asarray(mask, bool),
        np.asarray(attn_bias, np.float32),
    )
    rr = run_bass_kernel_spmd(
        nc, in_maps, list(range(8)), trace=TRACE,
        tmpdir=_CACHE.get("tmpdir"),
    )
    _CACHE["last_result"] = rr

    out = np.empty((B, H, N, D), np.float32)
    for c in range(8):
        bp, hq = divmod(c, 4)
        bs, hs = 2 * bp, 4 * hq
        oT = np.asarray(rr.results[c]["outT"])    # [NB,NH,D,N]
        out[bs:bs + NB, hs:hs + NH] = oT.transpose(0, 1, 3, 2)
    return out



# revision 3
# speedup vs baseline: 1.5044x; 1.5044x over previous
"""Sparse attention (B=4,H=16,N=2048,D=64) on 8 trn2 NeuronCores.

Sharding: core c = bp*4 + hq handles batches [2bp, 2bp+1] x heads [4hq..4hq+3].
Per (b,h), with host-precomputed mex = mask^T * exp(bias^T) / 16 streamed in:
  P^T = exp(K Q^T/8) * mex          (exp on ACT, mul on DVE)
  [O~ ; denom]^T = [V | 1]^T @ P^T  (accumulated fp32 in PSUM)
Device ships unnormalized [O~; denom] as fp16; host divides + transposes.

Flat software-pipelined loop over 128 key-tile-pair iterations per core:
S-matmuls for iter t run 2 iterations ahead of the O-matmuls (lag-2) so
the ACT engine (exp, the pacer at ~1.07us/tile) never starves behind
O-matmuls that wait on the DVE multiply.  Adjacent key tiles' S-matmuls
run concurrently in PE row groups 0-63/64-127 (q replicated in both
partition halves; k parity-packed so odd tiles live in rows 64-127).
"""

import numpy as np
import ml_dtypes

import concourse.bass as bass
from concourse import bacc
import concourse.mybir as mybir
import concourse.tile as tile
from concourse.bass_utils import run_bass_kernel_spmd

dt = mybir.dt
AF = mybir.ActivationFunctionType

B, H, N, D = 4, 16, 2048, 64
NB = 2    # batches per core
NH = 4    # heads per core
P = 128
NKT = N // P          # 16 key tiles
NKT2 = NKT // 2       # 8 key-tile pairs
QW = 512              # matmul free-dim (one PSUM bank of fp32)
SUPW = 1024           # S tile width / ACT width (2 PSUM banks)
NQP = N // SUPW       # 2 query supertiles
NPAIR = NB * NH       # 8 (b,h) pairs per core
T_ITER = NPAIR * NQP * NKT2   # 128 pipeline iterations
LAG = 2
SCALE = np.float32(1.0 / 16.0)   # folded into mex; cancels in normalization
TRACE = False

_CACHE = {}


def build_bass():
    nc = bacc.Bacc()
    # q^T scaled, replicated into both partition halves: [NB,NH,128,N]
    qT = nc.declare_dram_parameter("qT", [NB, NH, 2 * D, N], dt.float16, isOutput=False)
    # k^T parity-packed: rows 0-63 even key tiles, rows 64-127 odd: [NB,NH,128,N/2]
    kT = nc.declare_dram_parameter("kT", [NB, NH, 2 * D, N // 2], dt.float16, isOutput=False)
    # [V | 1] pre-tiled: [NB,NH,128,NKT*(D+1)]
    vA = nc.declare_dram_parameter("vA", [NB, NH, P, NKT * (D + 1)], dt.float16, isOutput=False)
    # mask*exp(bias)*SCALE, tiled per iteration: [T_ITER, 128, 2*SUPW]
    mex = nc.declare_dram_parameter("mex", [T_ITER, P, 2 * SUPW], dt.float16, isOutput=False)
    # unnormalized [O~; denom]^T per pair: [NB,NH,D+1,N]
    outU = nc.declare_dram_parameter("outU", [NB, NH, D + 1, N], dt.float16, isOutput=True)

    def sched(t):
        pair, r = divmod(t, NQP * NKT2)
        qp, ktp = divmod(r, NKT2)
        h, b = divmod(pair, NB)
        return pair, b, h, qp, ktp

    with tile.TileContext(nc) as tc:
        with (
            tc.tile_pool(name="qk", bufs=2) as qkpool,
            tc.tile_pool(name="vp", bufs=2) as vpool,
            tc.tile_pool(name="mex", bufs=4) as mpool,
            tc.tile_pool(name="pt", bufs=4) as ppool,
            tc.tile_pool(name="out", bufs=2) as opool_sb,
            tc.tile_pool(name="spsum", bufs=3, space="PSUM") as spool,
            tc.tile_pool(name="opsum", bufs=1, space="PSUM") as opool,
        ):
            qkv = [None] * NPAIR   # (qsb, ksb, vsb) per pair
            pts = [None] * T_ITER  # pt tile + indices for the O-side
            osum = [None]          # current opsum tiles

            def load_pair(p):
                h, b = divmod(p, NB)
                qt_ = qkpool.tile([2 * D, N], dt.float16, tag="q")
                kt_ = qkpool.tile([2 * D, N // 2], dt.float16, tag="k")
                vt_ = vpool.tile([P, NKT * (D + 1)], dt.float16, tag="v")
                nc.gpsimd.dma_start(qt_, qT[b, h])
                nc.gpsimd.dma_start(kt_, kT[b, h])
                nc.gpsimd.dma_start(vt_, vA[b, h])
                qkv[p] = (qt_, kt_, vt_)

            load_pair(0)
            for t in range(T_ITER + LAG):
                # ---------------- S side: S-matmuls, exp, mex multiply ----
                if t < T_ITER:
                    pair, b, h, qp, ktp = sched(t)
                    q0 = qp * SUPW
                    # prefetch next pair's q/k/v mid-way through this pair
                    if t % (NQP * NKT2) == NKT2 and pair + 1 < NPAIR:
                        load_pair(pair + 1)
                    qsb, ksb, vsb = qkv[pair]

                    mexsb = mpool.tile([P, 2 * SUPW], dt.float16, tag="mex")
                    nc.sync.dma_start(mexsb, mex[t])

                    pt = ppool.tile([P, 2 * SUPW], dt.float16, tag="pt")
                    ssups = []
                    for j in range(2):
                        ss = spool.tile([P, SUPW], dt.float32, tag="s",
                                        name=f"ssup{j}")
                        ssups.append(ss)
                    # adjacent key tiles (j=0,1) -> PE row groups 0/64,
                    # issued interleaved so each pair runs concurrently
                    ti = ktp  # column block in parity-packed k
                    for qi in range(2):
                        for j in range(2):
                            rg = j * D
                            nc.tensor.matmul(
                                ssups[j][:, qi * QW:(qi + 1) * QW],
                                ksb[rg:rg + D, ti * P:(ti + 1) * P],
                                qsb[rg:rg + D,
                                    q0 + qi * QW:q0 + (qi + 1) * QW],
                                start=True, stop=True,
                                tile_position=(rg, 0),
                            )
                    for j in range(2):
                        nc.scalar.activation(
                            pt[:, j * SUPW:(j + 1) * SUPW], ssups[j], AF.Exp)
                    nc.vector.tensor_mul(pt, pt, mexsb)
                    pts[t] = (pt, pair, b, h, qp, ktp)

                # ---------------- O side (lag-2): accumulate V~ @ P^T -----
                to = t - LAG
                if to >= 0:
                    pt, pair, b, h, qp, ktp = pts[to]
                    pts[to] = None
                    vsb = qkv[pair][2]
                    if ktp == 0:
                        osum[0] = [
                            opool.tile([D + 1, QW], dt.float32,
                                       tag=f"o{qi}", name=f"opsum{qi}")
                            for qi in range(2)
                        ]
                    opsum = osum[0]
                    for j in range(2):
                        kt = 2 * ktp + j
                        for qi in range(2):
                            nc.tensor.matmul(
                                opsum[qi],
                                vsb[:, kt * (D + 1):(kt + 1) * (D + 1)],
                                pt[:, j * SUPW + qi * QW:
                                   j * SUPW + (qi + 1) * QW],
                                start=(kt == 0), stop=(kt == NKT - 1),
                            )
                    if ktp == NKT2 - 1:
                        # drain unnormalized accumulators to SBUF as fp16
                        q0 = qp * SUPW
                        osb = opool_sb.tile([D + 1, SUPW], dt.float16,
                                            tag="osb")
                        for qi in range(2):
                            nc.vector.tensor_copy(
                                osb[:, qi * QW:(qi + 1) * QW], opsum[qi])
                        nc.gpsimd.dma_start(
                            outU[b, h, :, q0:q0 + SUPW], osb)
    nc.finalize()
    return nc


def make_in_maps(q, k, v, mask, attn_bias):
    scale = np.float32(D ** -0.5)
    qTf = (q.transpose(0, 1, 3, 2) * scale).astype(np.float16)   # [B,H,D,N]
    kTf = k.transpose(0, 1, 3, 2).astype(np.float16)             # [B,H,D,N]
    # parity-packed k^T: [B,H,2,D,N/2] -> [B,H,2D,N/2]
    kPar = np.ascontiguousarray(
        kTf.reshape(B, H, D, NKT2, 2, P).transpose(0, 1, 4, 2, 3, 5)
        .reshape(B, H, 2 * D, N // 2))
    vA = np.concatenate(
        [v, np.ones((B, H, N, 1), np.float32)], axis=-1
    ).astype(np.float16)                                         # [B,H,N,D+1]
    # pre-tiled: [B,H,P,NKT*(D+1)]
    vTile = np.ascontiguousarray(
        vA.reshape(B, H, NKT, P, D + 1).transpose(0, 1, 3, 2, 4)
        .reshape(B, H, P, NKT * (D + 1)))
    # transposed [key, query] views
    maskT = mask[:, 0].transpose(0, 2, 1)                        # [B,N,N] bool
    expbT = (np.exp(attn_bias[0].transpose(0, 2, 1),
                    dtype=np.float32) * SCALE).astype(np.float16)  # [H,N,N]

    in_maps = []
    for c in range(8):
        bp, hq = divmod(c, 4)
        bs, hs = 2 * bp, 4 * hq
        qrep = np.concatenate([qTf[bs:bs + NB, hs:hs + NH]] * 2, axis=2)
        # mex[t] = [128, 2*SUPW] for iteration t = ((h*NB+b)*NQP+qp)*NKT2+ktp
        # mex[t][p, j*SUPW+u] = maskT[key=(2ktp+j)*128+p, query=qp*SUPW+u]
        #                       * expbT[h][key, query] * SCALE
        mex = np.empty((T_ITER, P, 2 * SUPW), np.float16)
        for h in range(NH):
            for b in range(NB):
                m = np.where(maskT[bs + b], expbT[hs + h], np.float16(0))
                # [NKT2, 2, P, NQP, SUPW] -> [NQP, NKT2, P, 2, SUPW]
                mt = (m.reshape(NKT2, 2, P, NQP, SUPW)
                      .transpose(3, 0, 2, 1, 4)
                      .reshape(NQP, NKT2, P, 2 * SUPW))
                pairi = h * NB + b
                t0 = pairi * NQP * NKT2
                mex[t0:t0 + NQP * NKT2] = mt.reshape(
                    NQP * NKT2, P, 2 * SUPW)
        in_maps.append({
            "qT": np.ascontiguousarray(qrep),
            "kT": np.ascontiguousarray(kPar[bs:bs + NB, hs:hs + NH]),
            "vA": np.ascontiguousarray(vTile[bs:bs + NB, hs:hs + NH]),
            "mex": mex,
        })
    return in_maps


def kernel(q, k, v, mask, attn_bias):
    if "nc" not in _CACHE:
        _CACHE["nc"] = build_bass()
    nc = _CACHE["nc"]
    in_maps = make_in_maps(
        np.asarray(q, np.float32), np.asarray(k, np.float32),
        np.asarray(v, np.float32), np.asarray(mask, bool),
        np.asarray(attn_bias, np.float32),
    )
    rr = run_bass_kernel_spmd(
        nc, in_maps, list(range(8)), trace=TRACE,
        tmpdir=_CACHE.get("tmpdir"),
    )
    _CACHE["last_result"] = rr

    out = np.empty((B, H, N, D), np.float32)
    for c in range(8):
        bp, hq = divmod(c, 4)
        bs, hs = 2 * bp, 4 * hq
        oU = np.asarray(rr.results[c]["outU"]).astype(np.float32)  # [NB,NH,65,N]
        o = oU[:, :, :D, :] / oU[:, :, D:D + 1, :]
        out[bs:bs + NB, hs:hs + NH] = o.transpose(0, 1, 3, 2)
    return out


# revision 6
# speedup vs baseline: 1.5605x; 1.0373x over previous
"""Sparse attention (B=4,H=16,N=2048,D=64) on 8 trn2 NeuronCores.

Sharding: core c = bp*4 + hq handles batches [2bp, 2bp+1] x heads [4hq..4hq+3].
Per (b,h), with host-precomputed mex = mask^T * exp(bias^T) / 16 streamed in:
  P^T = exp(K Q^T/8) * mex          (exp on ACT, mul on DVE)
  [O~ ; denom]^T = [V | 1]^T @ P^T  (accumulated fp32 in PSUM)
Device ships unnormalized [O~; denom] as fp16; host divides + transposes.

Flat software-pipelined loop over 128 key-tile-pair iterations per core:
S-matmuls for iter t run 2 iterations ahead of the O-matmuls (lag-2) so
the ACT engine (exp, the pacer at ~1.07us/tile) never starves behind
O-matmuls that wait on the DVE multiply.  Adjacent key tiles' S-matmuls
run concurrently in PE row groups 0-63/64-127 (q replicated in both
partition halves; k parity-packed so odd tiles live in rows 64-127).
"""

import numpy as np
import ml_dtypes

import concourse.bass as bass
from concourse import bacc
import concourse.mybir as mybir
import concourse.tile as tile
from concourse.bass_utils import run_bass_kernel_spmd

dt = mybir.dt
AF = mybir.ActivationFunctionType

B, H, N, D = 4, 16, 2048, 64
NB = 2    # batches per core
NH = 4    # heads per core
P = 128
NKT = N // P          # 16 key tiles
NKT2 = NKT // 2       # 8 key-tile pairs
QW = 512              # matmul free-dim (one PSUM bank of fp32)
SUPW = 1024           # S tile width / ACT width (2 PSUM banks)
NQP = N // SUPW       # 2 query supertiles
NPAIR = NB * NH       # 8 (b,h) pairs per core
T_ITER = NPAIR * NQP * NKT2   # 128 pipeline iterations
LAG = 2
SCALE = np.float32(1.0 / 16.0)   # folded into mex; cancels in normalization
TRACE = False

_CACHE = {}


def build_bass():
    nc = bacc.Bacc()
    # q^T scaled, replicated into both partition halves: [NB,NH,128,N]
    qT = nc.declare_dram_parameter("qT", [NB, NH, 2 * D, N], dt.float16, isOutput=False)
    # k^T parity-packed: rows 0-63 even key tiles, rows 64-127 odd: [NB,NH,128,N/2]
    kT = nc.declare_dram_parameter("kT", [NB, NH, 2 * D, N // 2], dt.float16, isOutput=False)
    # [V | 1] pre-tiled: [NB,NH,128,NKT*(D+1)]
    vA = nc.declare_dram_parameter("vA", [NB, NH, P, NKT * (D + 1)], dt.float16, isOutput=False)
    # mask*exp(bias)*SCALE, tiled per iteration: [T_ITER, 128, 2*SUPW]
    mex = nc.declare_dram_parameter("mex", [T_ITER, P, 2 * SUPW], dt.float16, isOutput=False)
    # unnormalized [O~; denom]^T per pair: [NB,NH,D+1,N]
    outU = nc.declare_dram_parameter("outU", [NB, NH, D + 1, N], dt.float16, isOutput=True)

    def sched(t):
        pair, r = divmod(t, NQP * NKT2)
        qp, ktp = divmod(r, NKT2)
        h, b = divmod(pair, NB)
        return pair, b, h, qp, ktp

    with tile.TileContext(nc) as tc:
        with (
            tc.tile_pool(name="qk", bufs=2) as qkpool,
            tc.tile_pool(name="vp", bufs=2) as vpool,
            tc.tile_pool(name="mex", bufs=6) as mpool,
            tc.tile_pool(name="pt", bufs=6) as ppool,
            tc.tile_pool(name="out", bufs=2) as opool_sb,
            tc.tile_pool(name="spsum", bufs=3, space="PSUM") as spool,
            tc.tile_pool(name="opsum", bufs=1, space="PSUM") as opool,
        ):
            qkv = [None] * NPAIR   # (qsb, ksb, vsb) per pair
            pts = [None] * T_ITER  # pt tile + indices for the O-side
            osum = [None]          # current opsum tiles

            def load_pair(p):
                h, b = divmod(p, NB)
                qt_ = qkpool.tile([2 * D, N], dt.float16, tag="q")
                kt_ = qkpool.tile([2 * D, N // 2], dt.float16, tag="k")
                vt_ = vpool.tile([P, NKT * (D + 1)], dt.float16, tag="v")
                if p == 0:
                    # ramp: spread the critical q/k loads over idle engine
                    # queues so they aren't starved by the mex prefetch burst
                    # (scalar queue is safe here: no ACTIVATE issued yet)
                    nc.scalar.dma_start(kt_, kT[b, h])
                    nc.scalar.dma_start(qt_[:, :SUPW], qT[b, h, :, :SUPW])
                    nc.gpsimd.dma_start(qt_[:, SUPW:], qT[b, h, :, SUPW:])
                    nc.gpsimd.dma_start(vt_, vA[b, h])
                else:
                    nc.gpsimd.dma_start(qt_, qT[b, h])
                    nc.gpsimd.dma_start(kt_, kT[b, h])
                    nc.gpsimd.dma_start(vt_, vA[b, h])
                qkv[p] = (qt_, kt_, vt_)

            load_pair(0)
            for t in range(T_ITER + LAG):
                # ---------------- S side: S-matmuls, exp, mex multiply ----
                if t < T_ITER:
                    pair, b, h, qp, ktp = sched(t)
                    q0 = qp * SUPW
                    # prefetch next pair's q/k/v mid-way through this pair
                    if t % (NQP * NKT2) == NKT2 and pair + 1 < NPAIR:
                        load_pair(pair + 1)
                    qsb, ksb, vsb = qkv[pair]

                    mexsb = mpool.tile([P, 2 * SUPW], dt.float16, tag="mex")
                    nc.sync.dma_start(mexsb, mex[t])

                    pt = ppool.tile([P, 2 * SUPW], dt.float16, tag="pt")
                    ssups = []
                    for j in range(2):
                        ss = spool.tile([P, SUPW], dt.float32, tag="s",
                                        name=f"ssup{j}")
                        ssups.append(ss)
                    # adjacent key tiles (j=0,1) -> PE row groups 0/64,
                    # issued interleaved so each pair runs concurrently
                    ti = ktp  # column block in parity-packed k
                    for qi in range(2):
                        for j in range(2):
                            rg = j * D
                            nc.tensor.matmul(
                                ssups[j][:, qi * QW:(qi + 1) * QW],
                                ksb[rg:rg + D, ti * P:(ti + 1) * P],
                                qsb[rg:rg + D,
                                    q0 + qi * QW:q0 + (qi + 1) * QW],
                                start=True, stop=True,
                                tile_position=(rg, 0),
                            )
                    for j in range(2):
                        nc.scalar.activation(
                            pt[:, j * SUPW:(j + 1) * SUPW], ssups[j], AF.Exp)
                    nc.vector.tensor_mul(pt, pt, mexsb)
                    pts[t] = (pt, pair, b, h, qp, ktp)

                # ---------------- O side (lag-2): accumulate V~ @ P^T -----
                to = t - LAG
                if to >= 0:
                    pt, pair, b, h, qp, ktp = pts[to]
                    pts[to] = None
                    vsb = qkv[pair][2]
                    if ktp == 0:
                        osum[0] = [
                            opool.tile([D + 1, QW], dt.float32,
                                       tag=f"o{qi}", name=f"opsum{qi}")
                            for qi in range(2)
                        ]
                    opsum = osum[0]
                    for j in range(2):
                        kt = 2 * ktp + j
                        for qi in range(2):
                            nc.tensor.matmul(
                                opsum[qi],
                                vsb[:, kt * (D + 1):(kt + 1) * (D + 1)],
                                pt[:, j * SUPW + qi * QW:
                                   j * SUPW + (qi + 1) * QW],
                                start=(kt == 0), stop=(kt == NKT - 1),
                            )
                    if ktp == NKT2 - 1:
                        # drain unnormalized accumulators to SBUF as fp16
                        q0 = qp * SUPW
                        osb = opool_sb.tile([D + 1, SUPW], dt.float16,
                                            tag="osb")
                        for qi in range(2):
                            nc.vector.tensor_copy(
                                osb[:, qi * QW:(qi + 1) * QW], opsum[qi])
                        nc.gpsimd.dma_start(
                            outU[b, h, :, q0:q0 + SUPW], osb)
    nc.finalize()
    return nc


def make_in_maps(q, k, v, mask, attn_bias):
    scale = np.float32(D ** -0.5)
    qTf = (q.transpose(0, 1, 3, 2) * scale).astype(np.float16)   # [B,H,D,N]
    kTf = k.transpose(0, 1, 3, 2).astype(np.float16)             # [B,H,D,N]
    # parity-packed k^T: [B,H,2,D,N/2] -> [B,H,2D,N/2]
    kPar = np.ascontiguousarray(
        kTf.reshape(B, H, D, NKT2, 2, P).transpose(0, 1, 4, 2, 3, 5)
        .reshape(B, H, 2 * D, N // 2))
    vA = np.concatenate(
        [v, np.ones((B, H, N, 1), np.float32)], axis=-1
    ).astype(np.float16)                                         # [B,H,N,D+1]
    # pre-tiled: [B,H,P,NKT*(D+1)]
    vTile = np.ascontiguousarray(
        vA.reshape(B, H, NKT, P, D + 1).transpose(0, 1, 3, 2, 4)
        .reshape(B, H, P, NKT * (D + 1)))
    # transposed [key, query] views
    maskT = mask[:, 0].transpose(0, 2, 1)                        # [B,N,N] bool
    expbT = (np.exp(attn_bias[0].transpose(0, 2, 1),
                    dtype=np.float32) * SCALE).astype(np.float16)  # [H,N,N]

    in_maps = []
    for c in range(8):
        bp, hq = divmod(c, 4)
        bs, hs = 2 * bp, 4 * hq
        qrep = np.concatenate([qTf[bs:bs + NB, hs:hs + NH]] * 2, axis=2)
        # mex[t] = [128, 2*SUPW] for iteration t = ((h*NB+b)*NQP+qp)*NKT2+ktp
        # mex[t][p, j*SUPW+u] = maskT[key=(2ktp+j)*128+p, query=qp*SUPW+u]
        #                       * expbT[h][key, query] * SCALE
        mex = np.empty((T_ITER, P, 2 * SUPW), np.float16)
        for h in range(NH):
            for b in range(NB):
                m = np.where(maskT[bs + b], expbT[hs + h], np.float16(0))
                # [NKT2, 2, P, NQP, SUPW] -> [NQP, NKT2, P, 2, SUPW]
                mt = (m.reshape(NKT2, 2, P, NQP, SUPW)
                      .transpose(3, 0, 2, 1, 4)
                      .reshape(NQP, NKT2, P, 2 * SUPW))
                pairi = h * NB + b
                t0 = pairi * NQP * NKT2
                mex[t0:t0 + NQP * NKT2] = mt.reshape(
                    NQP * NKT2, P, 2 * SUPW)
        in_maps.append({
            "qT": np.ascontiguousarray(qrep),
            "kT": np.ascontiguousarray(kPar[bs:bs + NB, hs:hs + NH]),
            "vA": np.ascontiguousarray(vTile[bs:bs + NB, hs:hs + NH]),
            "mex": mex,
        })
    return in_maps


def kernel(q, k, v, mask, attn_bias):
    if "nc" not in _CACHE:
        _CACHE["nc"] = build_bass()
    nc = _CACHE["nc"]
    in_maps = make_in_maps(
        np.asarray(q, np.float32), np.asarray(k, np.float32),
        np.asarray(v, np.float32), np.asarray(mask, bool),
        np.asarray(attn_bias, np.float32),
    )
    rr = run_bass_kernel_spmd(
        nc, in_maps, list(range(8)), trace=TRACE,
        tmpdir=_CACHE.get("tmpdir"),
    )
    _CACHE["last_result"] = rr

    out = np.empty((B, H, N, D), np.float32)
    for c in range(8):
        bp, hq = divmod(c, 4)
        bs, hs = 2 * bp, 4 * hq
        oU = np.asarray(rr.results[c]["outU"]).astype(np.float32)  # [NB,NH,65,N]
        o = oU[:, :, :D, :] / oU[:, :, D:D + 1, :]
        out[bs:bs + NB, hs:hs + NH] = o.transpose(0, 1, 3, 2)
    return out


# revision 8
# speedup vs baseline: 1.5946x; 1.0218x over previous
"""Sparse attention (B=4,H=16,N=2048,D=64) on 8 trn2 NeuronCores.

Sharding: core c = bp*4 + hq handles batches [2bp, 2bp+1] x heads [4hq..4hq+3].
Per (b,h), with host-precomputed mex = mask^T * exp(bias^T) / 16 streamed in:
  P^T = exp(K Q^T/8) * mex          (exp on ACT, mul on DVE)
  [O~ ; denom]^T = [V | 1]^T @ P^T  (accumulated fp32 in PSUM)
Device ships unnormalized [O~; denom] as fp16; host divides + transposes.

Flat software-pipelined loop over 128 key-tile-pair iterations per core:
S-matmuls for iter t run 2 iterations ahead of the O-matmuls (lag-2) so
the ACT engine (exp, the pacer at ~1.07us/tile) never starves behind
O-matmuls that wait on the DVE multiply.  Adjacent key tiles' S-matmuls
run concurrently in PE row groups 0-63/64-127 (q replicated in both
partition halves; k parity-packed so odd tiles live in rows 64-127).
"""

import numpy as np
import ml_dtypes

import concourse.bass as bass
from concourse import bacc
import concourse.mybir as mybir
import concourse.tile as tile
from concourse.bass_utils import run_bass_kernel_spmd

dt = mybir.dt
AF = mybir.ActivationFunctionType

B, H, N, D = 4, 16, 2048, 64
NB = 2    # batches per core
NH = 4    # heads per core
P = 128
NKT = N // P          # 16 key tiles
NKT2 = NKT // 2       # 8 key-tile pairs
QW = 512              # matmul free-dim (one PSUM bank of fp32)
SUPW = 1024           # S tile width / ACT width (2 PSUM banks)
NQP = N // SUPW       # 2 query supertiles
NPAIR = NB * NH       # 8 (b,h) pairs per core
T_ITER = NPAIR * NQP * NKT2   # 128 pipeline iterations
LAG = 2
SCALE = np.float32(1.0 / 16.0)   # folded into mex; cancels in normalization
TRACE = False

_CACHE = {}


def build_bass():
    nc = bacc.Bacc()
    # q^T scaled, replicated into both partition halves: [NB,NH,128,N]
    qT = nc.declare_dram_parameter("qT", [NB, NH, 2 * D, N], dt.float16, isOutput=False)
    # k^T parity-packed: rows 0-63 even key tiles, rows 64-127 odd: [NB,NH,128,N/2]
    kT = nc.declare_dram_parameter("kT", [NB, NH, 2 * D, N // 2], dt.float16, isOutput=False)
    # [V | 1] pre-tiled: [NB,NH,128,NKT*(D+1)]
    vA = nc.declare_dram_parameter("vA", [NB, NH, P, NKT * (D + 1)], dt.float16, isOutput=False)
    # mask*exp(bias)*SCALE, tiled per iteration: [T_ITER, 128, 2*SUPW]
    mex = nc.declare_dram_parameter("mex", [T_ITER, P, 2 * SUPW], dt.float16, isOutput=False)
    # unnormalized [O~; denom]^T per pair: [NB,NH,D+1,N]
    outU = nc.declare_dram_parameter("outU", [NB, NH, D + 1, N], dt.float16, isOutput=True)

    def sched(t):
        pair, r = divmod(t, NQP * NKT2)
        qp, ktp = divmod(r, NKT2)
        h, b = divmod(pair, NB)
        return pair, b, h, qp, ktp

    with tile.TileContext(nc) as tc:
        with (
            tc.tile_pool(name="qk", bufs=2) as qkpool,
            tc.tile_pool(name="vp", bufs=2) as vpool,
            tc.tile_pool(name="mex", bufs=6) as mpool,
            tc.tile_pool(name="pt", bufs=6) as ppool,
            tc.tile_pool(name="out", bufs=2) as opool_sb,
            tc.tile_pool(name="spsum", bufs=3, space="PSUM") as spool,
            tc.tile_pool(name="opsum", bufs=1, space="PSUM") as opool,
        ):
            qkv = [None] * NPAIR   # (qsb, ksb, vsb) per pair
            pts = [None] * T_ITER  # pt tile + indices for the O-side
            osum = [None]          # current opsum tiles

            def load_pair(p):
                h, b = divmod(p, NB)
                qt_ = qkpool.tile([2 * D, N], dt.float16, tag="q")
                kt_ = qkpool.tile([2 * D, N // 2], dt.float16, tag="k")
                vt_ = vpool.tile([P, NKT * (D + 1)], dt.float16, tag="v")
                if p == 0:
                    # ramp: spread the critical q/k loads over idle engine
                    # queues so they aren't starved by the mex prefetch burst.
                    # q goes at the HEAD of the sync queue (before all mex),
                    # k alone on the scalar queue (safe: no ACTIVATE yet).
                    nc.sync.dma_start(qt_, qT[b, h])
                    nc.scalar.dma_start(kt_, kT[b, h])
                    nc.gpsimd.dma_start(vt_, vA[b, h])
                else:
                    nc.gpsimd.dma_start(qt_, qT[b, h])
                    nc.gpsimd.dma_start(kt_, kT[b, h])
                    nc.gpsimd.dma_start(vt_, vA[b, h])
                qkv[p] = (qt_, kt_, vt_)

            load_pair(0)
            for t in range(T_ITER + LAG):
                # ---------------- S side: S-matmuls, exp, mex multiply ----
                if t < T_ITER:
                    pair, b, h, qp, ktp = sched(t)
                    q0 = qp * SUPW
                    # prefetch next pair's q/k/v mid-way through this pair
                    if t % (NQP * NKT2) == NKT2 and pair + 1 < NPAIR:
                        load_pair(pair + 1)
                    qsb, ksb, vsb = qkv[pair]

                    mexsb = mpool.tile([P, 2 * SUPW], dt.float16, tag="mex")
                    nc.sync.dma_start(mexsb, mex[t])

                    pt = ppool.tile([P, 2 * SUPW], dt.float16, tag="pt")
                    ssups = []
                    for j in range(2):
                        ss = spool.tile([P, SUPW], dt.float32, tag="s",
                                        name=f"ssup{j}")
                        ssups.append(ss)
                    # adjacent key tiles (j=0,1) -> PE row groups 0/64,
                    # issued interleaved so each pair runs concurrently
                    ti = ktp  # column block in parity-packed k
                    for j in range(2):
                        for qi in range(2):
                            rg = j * D
                            nc.tensor.matmul(
                                ssups[j][:, qi * QW:(qi + 1) * QW],
                                ksb[rg:rg + D, ti * P:(ti + 1) * P],
                                qsb[rg:rg + D,
                                    q0 + qi * QW:q0 + (qi + 1) * QW],
                                start=True, stop=True,
                                tile_position=(rg, 0),
                            )
                    for j in range(2):
                        nc.scalar.activation(
                            pt[:, j * SUPW:(j + 1) * SUPW], ssups[j], AF.Exp)
                    nc.vector.tensor_mul(pt, pt, mexsb)
                    pts[t] = (pt, pair, b, h, qp, ktp)

                # ---------------- O side (lag-2): accumulate V~ @ P^T -----
                to = t - LAG
                if to >= 0:
                    pt, pair, b, h, qp, ktp = pts[to]
                    pts[to] = None
                    vsb = qkv[pair][2]
                    if ktp == 0:
                        osum[0] = [
                            opool.tile([D + 1, QW], dt.float32,
                                       tag=f"o{qi}", name=f"opsum{qi}")
                            for qi in range(2)
                        ]
                    opsum = osum[0]
                    for j in range(2):
                        kt = 2 * ktp + j
                        for qi in range(2):
                            nc.tensor.matmul(
                                opsum[qi],
                                vsb[:, kt * (D + 1):(kt + 1) * (D + 1)],
                                pt[:, j * SUPW + qi * QW:
                                   j * SUPW + (qi + 1) * QW],
                                start=(kt == 0), stop=(kt == NKT - 1),
                            )
                    if ktp == NKT2 - 1:
                        # drain unnormalized accumulators to SBUF as fp16
                        q0 = qp * SUPW
                        osb = opool_sb.tile([D + 1, SUPW], dt.float16,
                                            tag="osb")
                        for qi in range(2):
                            nc.vector.tensor_copy(
                                osb[:, qi * QW:(qi + 1) * QW], opsum[qi])
                        nc.gpsimd.dma_start(
                            outU[b, h, :, q0:q0 + SUPW], osb)
    nc.finalize()
    return nc


def make_in_maps(q, k, v, mask, attn_bias):
    scale = np.float32(D ** -0.5)
    qTf = (q.transpose(0, 1, 3, 2) * scale).astype(np.float16)   # [B,H,D,N]
    kTf = k.transpose(0, 1, 3, 2).astype(np.float16)             # [B,H,D,N]
    # parity-packed k^T: [B,H,2,D,N/2] -> [B,H,2D,N/2]
    kPar = np.ascontiguousarray(
        kTf.reshape(B, H, D, NKT2, 2, P).transpose(0, 1, 4, 2, 3, 5)
        .reshape(B, H, 2 * D, N // 2))
    vA = np.concatenate(
        [v, np.ones((B, H, N, 1), np.float32)], axis=-1
    ).astype(np.float16)                                         # [B,H,N,D+1]
    # pre-tiled: [B,H,P,NKT*(D+1)]
    vTile = np.ascontiguousarray(
        vA.reshape(B, H, NKT, P, D + 1).transpose(0, 1, 3, 2, 4)
        .reshape(B, H, P, NKT * (D + 1)))
    # transposed [key, query] views
    maskT = mask[:, 0].transpose(0, 2, 1)                        # [B,N,N] bool
    expbT = (np.exp(attn_bias[0].transpose(0, 2, 1),
                    dtype=np.float32) * SCALE).astype(np.float16)  # [H,N,N]

    in_maps = []
    for c in range(8):
        bp, hq = divmod(c, 4)
        bs, hs = 2 * bp, 4 * hq
        qrep = np.concatenate([qTf[bs:bs + NB, hs:hs + NH]] * 2, axis=2)
        # mex[t] = [128, 2*SUPW] for iteration t = ((h*NB+b)*NQP+qp)*NKT2+ktp
        # mex[t][p, j*SUPW+u] = maskT[key=(2ktp+j)*128+p, query=qp*SUPW+u]
        #                       * expbT[h][key, query] * SCALE
        mex = np.empty((T_ITER, P, 2 * SUPW), np.float16)
        for h in range(NH):
            for b in range(NB):
                m = np.where(maskT[bs + b], expbT[hs + h], np.float16(0))
                # [NKT2, 2, P, NQP, SUPW] -> [NQP, NKT2, P, 2, SUPW]
                mt = (m.reshape(NKT2, 2, P, NQP, SUPW)
                      .transpose(3, 0, 2, 1, 4)
                      .reshape(NQP, NKT2, P, 2 * SUPW))
                pairi = h * NB + b
                t0 = pairi * NQP * NKT2
                mex[t0:t0 + NQP * NKT2] = mt.reshape(
                    NQP * NKT2, P, 2 * SUPW)
        in_maps.append({
            "qT": np.ascontiguousarray(qrep),
            "kT": np.ascontiguousarray(kPar[bs:bs + NB, hs:hs + NH]),
            "vA": np.ascontiguousarray(vTile[bs:bs + NB, hs:hs + NH]),
            "mex": mex,
        })
    return in_maps


def kernel(q, k, v, mask, attn_bias):
    if "nc" not in _CACHE:
        _CACHE["nc"] = build_bass()
    nc = _CACHE["nc"]
    in_maps = make_in_maps(
        np.asarray(q, np.float32), np.asarray(k, np.float32),
        np.asarray(v, np.float32), np.asarray(mask, bool),
        np.asarray(attn_bias, np.float32),
    )
    rr = run_bass_kernel_spmd(
        nc, in_maps, list(range(8)), trace=TRACE,
        tmpdir=_CACHE.get("tmpdir"),
    )
    _CACHE["last_result"] = rr

    out = np.empty((B, H, N, D), np.float32)
    for c in range(8):
        bp, hq = divmod(c, 4)
        bs, hs = 2 * bp, 4 * hq
        oU = np.asarray(rr.results[c]["outU"]).astype(np.float32)  # [NB,NH,65,N]
        o = oU[:, :, :D, :] / oU[:, :, D:D + 1, :]
        out[bs:bs + NB, hs:hs + NH] = o.transpose(0, 1, 3, 2)
    return out
